# revision 5
# baseline (speedup 1.0000x reference)
"""MesoNet GNN message-passing kernel for 8 Trainium2 NeuronCores.

Sharding: groups/atoms/edges partitioned contiguously across cores; incidence
items and edges sorted by segment and laid out in 128-segment chunks with
fixed per-chunk tile capacity. Segment softmax/sums via one-hot selection
matmuls (PSUM accumulation). Cross-core row access via two AllGathers
(m table for edges, xg table for the groups->atoms Set2Set).
"""
import os
import numpy as np

import concourse.bass as bass
import concourse.bacc as bacc
import concourse.mybir as mybir
import concourse.tile as tile
import concourse.bass_utils as bass_utils
from concourse.masks import make_identity

F32 = mybir.dt.float32
I32 = mybir.dt.int32
AF = mybir.ActivationFunctionType
OP = mybir.AluOpType

NCORES = 8
D = 160
DC = D + 1          # row width with ones column
PAD_LC = 200.0      # out-of-range one-hot column marker for padding items

LAST_EXEC_NS = [None]


class Cfg:
    def __init__(self, NA, G, NI, E):
        self.NA, self.G, self.NI, self.E = NA, G, NI, E
        self.SG = G // NCORES
        self.SGP = ((self.SG + 127) // 128) * 128
        self.KG = self.SGP // 128
        self.SA = NA // NCORES
        self.SAP = ((self.SA + 127) // 128) * 128
        self.KA = self.SAP // 128
        self.KE = self.KG
        self.TG = None
        self.TA = None
        self.TE = None

    def colchunks(self, n):
        out, s = [], 0
        while s < n:
            w = min(512, n - s)
            out.append((s, w))
            s += w
        return out


def _sigmoid(x):
    return 1.0 / (1.0 + np.exp(-x))


def _f32(x):
    return np.ascontiguousarray(np.asarray(x, np.float32))


def _split_hi_lo(M):
    M = _f32(M)
    assert M.shape[0] == D
    return np.ascontiguousarray(M[:128]), np.ascontiguousarray(M[128:])


def _chunked_items(seg_local, nchunks, cap, values, lc_local):
    T = cap // 128
    idx = np.zeros((nchunks, 128, T), np.int32)
    lc = np.full((nchunks, 128, T), PAD_LC, np.float32)
    ck = seg_local // 128
    order = np.argsort(ck, kind="stable")
    ck_s, v_s, l_s = ck[order], values[order], lc_local[order]
    bounds = np.searchsorted(ck_s, np.arange(nchunks + 1))
    for k in range(nchunks):
        a, b = bounds[k], bounds[k + 1]
        n = b - a
        assert n <= cap, (n, cap)
        s = np.arange(n)
        idx[k, s % 128, s // 128] = v_s[a:b]
        lc[k, s % 128, s // 128] = l_s[a:b]
    return idx, lc


def prepare(inputs, cfg):
    NA, G = cfg.NA, cfg.G
    SG, SGP, KG = cfg.SG, cfg.SGP, cfg.KG
    SA, SAP, KA = cfg.SA, cfg.SAP, cfg.KA

    x_atom = _f32(inputs["x_atom"])
    x_group = _f32(inputs["x_group"])
    cond_atom = _f32(inputs["cond_atom"])
    atom_idx = np.asarray(inputs["atom_idx"]).astype(np.int64)
    group_idx = np.asarray(inputs["group_idx"]).astype(np.int64)
    src, dst = [np.asarray(v).astype(np.int64)
                for v in inputs["edge_index_group"]]

    aw, ab = [_f32(v) for v in inputs["a_proj"]]
    gw, gb = [_f32(v) for v in inputs["g_proj"]]
    mw, mb = [_f32(v) for v in inputs["merge_a2g"]]
    pw, pb = [_f32(v) for v in inputs["g_proj_to_a"]]
    msg_w, msg_b, self_w, self_b, att = [_f32(v) for v in inputs["gcn"]]
    fg = [_f32(v) for v in inputs["film_gamma"]]
    fb = [_f32(v) for v in inputs["film_beta"]]

    def s2s_fold(params, absorb):
        W_ih, W_hh, b_ih, b_hh = [_f32(v) for v in params]
        Wq = W_ih[:, :D] + W_hh
        Wr = W_ih[:, D:]
        b = b_ih + b_hh
        i1, f1, g1, o1 = np.split(b, 4)
        c1 = _sigmoid(i1) * np.tanh(g1)
        q1 = _sigmoid(o1) * np.tanh(c1)
        be2 = b + Wq @ q1
        if absorb is not None:
            A, bb = absorb
            Wre = Wr @ A
            be2 = be2 + Wr @ bb
            q1t = A.T @ q1
            eoff1 = float(bb @ q1)
        else:
            Wre = Wr
            q1t = q1
            eoff1 = 0.0
        return dict(Wre=Wre, be2=be2, c1=c1, q1row=np.concatenate(
            [q1t, [eoff1]]).astype(np.float32)[None, :])

    sA = s2s_fold(inputs["s2s_a2g"], (aw, ab))
    sG = s2s_fold(inputs["s2s_g2a"], None)

    def remap_g(g):
        return (g // SG) * SGP + (g % SG)

    core_g = group_idx // SG
    lg = group_idx - core_g * SG
    fills_g = np.bincount(core_g * KG + (lg // 128), minlength=NCORES * KG)
    cfg.TG = int((int(fills_g.max()) + 127) // 128)

    core_a = atom_idx // SA
    la = atom_idx - core_a * SA
    fills_a = np.bincount(core_a * KA + (la // 128), minlength=NCORES * KA)
    cfg.TA = int((int(fills_a.max()) + 127) // 128)

    core_e = dst // SG
    le = dst - core_e * SG
    fills_e = np.bincount(core_e * cfg.KE + (le // 128),
                          minlength=NCORES * cfg.KE)
    cfg.TE = int((int(fills_e.max()) + 127) // 128)

    xat = np.concatenate([x_atom, np.ones((NA, 1), np.float32), cond_atom], 1)

    rep = {"xat": xat, "q1row_a2g": sA["q1row"], "q1row_g2a": sG["q1row"]}

    def add_hilo(name, M):
        hi, lo = _split_hi_lo(M)
        rep[name + "_hi"] = hi
        rep[name + "_lo"] = lo

    for g in range(4):
        add_hilo(f"WreTA{g}", sA["Wre"][g * D:(g + 1) * D].T)
        add_hilo(f"WreTG{g}", sG["Wre"][g * D:(g + 1) * D].T)
    for nm, M in [("aw", aw), ("awT", aw.T), ("mwqT", mw[:, :D].T),
                  ("mwrT", mw[:, D:].T), ("fgW1T", fg[0].T), ("fgW2T", fg[2].T),
                  ("fbW1T", fb[0].T), ("fbW2T", fb[2].T), ("msgT", msg_w.T),
                  ("selfT", self_w.T), ("pwqT", pw[:, :D].T),
                  ("pwrT", pw[:, D:].T)]:
        add_hilo(nm, M)

    def col(name, v):
        v = _f32(v).reshape(-1, 1)
        if v.shape[0] == D:
            rep[name + "_hi"] = _f32(v[:128])
            rep[name + "_lo"] = _f32(v[128:])
        else:
            rep[name] = v

    for g, nm in enumerate("ifgo"):
        col(f"be2A_{nm}", sA["be2"][g * D:(g + 1) * D])
        col(f"be2G_{nm}", sG["be2"][g * D:(g + 1) * D])
    col("c1A", sA["c1"])
    col("c1G", sG["c1"])
    col("ab", ab)
    col("mb", mb)
    col("fg_b1", fg[1]); col("fg_b2", fg[3])
    col("fb_b1", fb[1]); col("fb_b2", fb[3])
    col("msg_b", msg_b); col("self_b", self_b)
    col("att", att)
    col("gb", gb)
    col("pb", pb)
    rep["gwT"] = _f32(gw.T)

    in_maps = []
    for c in range(NCORES):
        m = core_g == c
        ai, lcg = _chunked_items(lg[m], KG, cfg.TG * 128,
                                 atom_idx[m].astype(np.int32),
                                 (lg[m] % 128).astype(np.float32))
        m = core_a == c
        gi, lca = _chunked_items(la[m], KA, cfg.TA * 128,
                                 remap_g(group_idx[m]).astype(np.int32),
                                 (la[m] % 128).astype(np.float32))
        m = core_e == c
        si, lce = _chunked_items(le[m], cfg.KE, cfg.TE * 128,
                                 remap_g(src[m]).astype(np.int32),
                                 (le[m] % 128).astype(np.float32))

        cntg = np.bincount(lg[core_g == c], minlength=SGP).astype(np.float32)
        cnta = np.bincount(la[core_a == c], minlength=SAP).astype(np.float32)

        xg_sl = np.zeros((SGP, 40), np.float32)
        xg_sl[:SG] = x_group[c * SG:(c + 1) * SG, :40]
        xa_sl = np.zeros((SAP, D), np.float32)
        xa_sl[:SA] = x_atom[c * SA:(c + 1) * SA]
        xaT = np.ascontiguousarray(xa_sl.T)

        core = dict(rep)
        core.update({
            "a2g_idx": ai, "a2g_lc": lcg,
            "g2a_idx": gi, "g2a_lc": lca,
            "edge_idx": si, "edge_lc": lce,
            "invcntg": (1.0 / np.maximum(cntg, 1.0)).reshape(-1, 1),
            "maskg_row": (cntg > 0).astype(np.float32)[None, :],
            "maska_row": (cnta > 0).astype(np.float32)[None, :],
            "xgroupT": np.ascontiguousarray(xg_sl.T),
            "xatomT_hi": _f32(xaT[:128]),
            "xatomT_lo": _f32(xaT[128:]),
        })
        in_maps.append(core)
    return in_maps


# ---------------------------------------------------------------------------
# device program
# ---------------------------------------------------------------------------

def build(nc, cfg):
    KG, KA, KE = cfg.KG, cfg.KA, cfg.KE
    TG, TA, TE = cfg.TG, cfg.TA, cfg.TE
    SGP, SAP = cfg.SGP, cfg.SAP

    def din(name, shape, dt=F32):
        return nc.dram_tensor(name, list(shape), dt, kind="ExternalInput")

    xat = din("xat", [cfg.NA, 2 * D + 1])
    q1A = din("q1row_a2g", [1, DC])
    q1G = din("q1row_g2a", [1, DC])
    P = {}
    hilo = ([f"WreTA{g}" for g in range(4)] + [f"WreTG{g}" for g in range(4)]
            + ["aw", "awT", "mwqT", "mwrT", "fgW1T", "fgW2T", "fbW1T",
               "fbW2T", "msgT", "selfT", "pwqT", "pwrT"])
    wid = {"msgT": 80, "selfT": 80}
    for nm in hilo:
        w = wid.get(nm, D)
        P[nm + "_hi"] = din(nm + "_hi", [128, w])
        P[nm + "_lo"] = din(nm + "_lo", [32, w])
    for nm in ["be2A_i", "be2A_f", "be2A_g", "be2A_o", "be2G_i", "be2G_f",
               "be2G_g", "be2G_o", "c1A", "c1G", "ab", "mb", "fg_b1", "fg_b2",
               "fb_b1", "fb_b2", "pb"]:
        P[nm + "_hi"] = din(nm + "_hi", [128, 1])
        P[nm + "_lo"] = din(nm + "_lo", [32, 1])
    for nm, h in [("msg_b", 80), ("self_b", 80), ("att", 80), ("gb", 80)]:
        P[nm] = din(nm, [h, 1])
    P["gwT"] = din("gwT", [40, 80])

    a2g_idx = din("a2g_idx", [KG, 128, TG], I32)
    a2g_lc = din("a2g_lc", [KG, 128, TG])
    g2a_idx = din("g2a_idx", [KA, 128, TA], I32)
    g2a_lc = din("g2a_lc", [KA, 128, TA])
    edge_idx = din("edge_idx", [KE, 128, TE], I32)
    edge_lc = din("edge_lc", [KE, 128, TE])
    invcntg = din("invcntg", [SGP, 1])
    maskg_row = din("maskg_row", [1, SGP])
    maska_row = din("maska_row", [1, SAP])
    xgroupT = din("xgroupT", [40, SGP])
    xatomT_hi = din("xatomT_hi", [128, SAP])
    xatomT_lo = din("xatomT_lo", [32, SAP])

    xg_out = nc.dram_tensor("xg_out", [SGP, D], F32, kind="ExternalOutput")
    xaT_hi_o = nc.dram_tensor("xaT_hi", [128, SAP], F32, kind="ExternalOutput")
    xaT_lo_o = nc.dram_tensor("xaT_lo", [32, SAP], F32, kind="ExternalOutput")

    def scratch(name, shape, **kw):
        return nc.dram_tensor(name, list(shape), F32, kind="Internal", **kw)

    xpackA = scratch("xpackA", [KG, 128, TG * DC])
    xpackG = scratch("xpackG", [KA, 128, TA * DC])
    r1TA_hi = scratch("r1TA_hi", [128, SGP]); r1TA_lo = scratch("r1TA_lo", [32, SGP])
    r2TA_hi = scratch("r2TA_hi", [128, SGP]); r2TA_lo = scratch("r2TA_lo", [32, SGP])
    r1TG_hi = scratch("r1TG_hi", [128, SAP]); r1TG_lo = scratch("r1TG_lo", [32, SAP])
    r2TG_hi = scratch("r2TG_hi", [128, SAP]); r2TG_lo = scratch("r2TG_lo", [32, SAP])
    condT_hi = scratch("condT_hi", [128, SGP]); condT_lo = scratch("condT_lo", [32, SGP])
    h2TA_hi = scratch("h2TA_hi", [128, SGP]); h2TA_lo = scratch("h2TA_lo", [32, SGP])
    h2TG_hi = scratch("h2TG_hi", [128, SAP]); h2TG_lo = scratch("h2TG_lo", [32, SAP])
    qnA = scratch("qnA", [KG, 128, DC])
    qnG = scratch("qnG", [KA, 128, DC])
    selfT_d = scratch("selfT_d", [80, SGP])
    xg0T_d = scratch("xg0T_d", [80, SGP])
    m_ag_in = scratch("m_ag_in", [SGP, 82])
    m_full = scratch("m_full", [NCORES * SGP, 82], addr_space="Shared")
    xg_ag_in = scratch("xg_ag_in", [SGP, DC])
    xg_full = scratch("xg_full", [NCORES * SGP, DC], addr_space="Shared")

    with tile.TileContext(nc) as tc:
        with tc.tile_pool(name="pp", bufs=1) as pp, \
             tc.tile_pool(name="sb1", bufs=1) as sb1, \
             tc.tile_pool(name="hot", bufs=2) as hot, \
             tc.tile_pool(name="big", bufs=2) as big, \
             tc.tile_pool(name="psa", bufs=1, space="PSUM") as psa, \
             tc.tile_pool(name="psw", bufs=3, space="PSUM") as psw, \
             tc.tile_pool(name="psm", bufs=3, space="PSUM") as psm:

            iota = pp.tile([128, 128], F32)
            nc.gpsimd.iota(iota[:], pattern=[[1, 128]], base=0,
                           channel_multiplier=0,
                           allow_small_or_imprecise_dtypes=True)
            ident = pp.tile([128, 128], F32)
            make_identity(nc, ident[:])
            q1bA = pp.tile([128, DC], F32)
            trow = pp.tile([1, DC], F32, tag="trow1")
            nc.sync.dma_start(out=trow[:], in_=q1A[:, :])
            nc.gpsimd.partition_broadcast(q1bA[:], trow[:1, :])
            q1bG = pp.tile([128, DC], F32)
            trow2 = pp.tile([1, DC], F32, tag="trow2")
            nc.sync.dma_start(out=trow2[:], in_=q1G[:, :])
            nc.gpsimd.partition_broadcast(q1bG[:], trow2[:1, :])

            prm = {}
            for nm, t in P.items():
                prm[nm] = pp.tile([t.shape[0], t.shape[1]], F32, tag=f"p_{nm}", name=f"p_{nm}")
                nc.sync.dma_start(out=prm[nm][:], in_=t[:, :])

            def trans_to(dst_hi, dst_lo, src_sb, k):
                t1 = psw.tile([128, 128], F32, tag="w", space="PSUM")
                nc.tensor.transpose(out=t1[:], in_=src_sb[:, :128],
                                    identity=ident[:])
                s1 = hot.tile([128, 128], F32, tag="tcp1")
                nc.vector.tensor_copy(out=s1[:], in_=t1[:])
                nc.sync.dma_start(out=dst_hi[k * 128:(k + 1) * 128, :]
                                  if False else dst_hi[:, k * 128:(k + 1) * 128],
                                  in_=s1[:])
                t2 = psw.tile([32, 128], F32, tag="w", space="PSUM")
                nc.tensor.transpose(out=t2[:], in_=src_sb[:, 128:160],
                                    identity=ident[:])
                s2 = hot.tile([32, 128], F32, tag="tcp2")
                nc.vector.tensor_copy(out=s2[:], in_=t2[:])
                nc.sync.dma_start(out=dst_lo[:, k * 128:(k + 1) * 128],
                                  in_=s2[:])

            # ---------------- item pass ----------------
            def item_pass(K, T, idx_t, lc_t, table, q1b, pack, step,
                          qn_d, r_hi, r_lo, cond_pass, gather_w):
                for k in range(K):
                    lc = hot.tile([128, T], F32, tag="lc")
                    nc.sync.dma_start(out=lc[:], in_=lc_t[k])
                    xtw = gather_w if step == 1 else DC
                    xtc = big.tile([128, T, xtw], F32, tag=f"xt{step}{int(cond_pass)}")
                    if step == 1:
                        idx = hot.tile([128, T], I32, tag="idx")
                        nc.sync.dma_start(out=idx[:], in_=idx_t[k])
                        for t in range(T):
                            nc.gpsimd.indirect_dma_start(
                                out=xtc[:, t, :], out_offset=None,
                                in_=table[:, :],
                                in_offset=bass.IndirectOffsetOnAxis(
                                    ap=idx[:, t:t + 1], axis=0))
                        nc.sync.dma_start(out=pack[k], in_=xtc[:, :, :DC])
                        qn = None
                    else:
                        nc.sync.dma_start(out=xtc[:, :, :], in_=pack[k])
                        qn = hot.tile([128, DC], F32, tag="qn")
                        nc.sync.dma_start(out=qn[:], in_=qn_d[k])
                    selbig = big.tile([128, T * 128], F32, tag="selb")
                    ebuf = hot.tile([128, T], F32, tag="ebuf")
                    scr = hot.tile([128, DC], F32, tag="scr")
                    for t in range(T):
                        nc.vector.tensor_scalar(
                            out=selbig[:, t * 128:(t + 1) * 128], in0=iota[:],
                            scalar1=lc[:, t:t + 1], scalar2=None,
                            op0=OP.is_equal)
                        if step == 1:
                            qin = q1b[:]
                        else:
                            stp = psw.tile([128, 128], F32, tag="w", space="PSUM")
                            nc.tensor.transpose(
                                out=stp[:],
                                in_=selbig[:, t * 128:(t + 1) * 128],
                                identity=ident[:])
                            sts = hot.tile([128, 128], F32, tag="selTs")
                            nc.vector.tensor_copy(out=sts[:], in_=stp[:])
                            qit = psw.tile([128, DC], F32, tag="w", space="PSUM")
                            nc.tensor.matmul(qit[:], lhsT=sts[:], rhs=qn[:],
                                             start=True, stop=True)
                            qin = qit[:]
                        nc.vector.scalar_tensor_tensor(
                            out=scr[:], in0=xtc[:, t, :DC], scalar=1.0,
                            in1=qin, op0=OP.mult, op1=OP.mult,
                            accum_out=ebuf[:, t:t + 1])
                    ea = hot.tile([128, T], F32, tag="ea")
                    nc.scalar.activation(ea[:], ebuf[:], AF.Exp)
                    pr = psa.tile([128, DC], F32, tag="pr", space="PSUM")
                    if cond_pass:
                        pc = psa.tile([128, D], F32, tag="pc", space="PSUM")
                    for t in range(T):
                        if cond_pass:
                            nc.tensor.matmul(
                                pc[:], lhsT=selbig[:, t * 128:(t + 1) * 128],
                                rhs=xtc[:, t, DC:DC + D],
                                start=(t == 0), stop=(t == T - 1))
                        nc.vector.tensor_scalar(
                            out=selbig[:, t * 128:(t + 1) * 128],
                            in0=selbig[:, t * 128:(t + 1) * 128],
                            scalar1=ea[:, t:t + 1], scalar2=None, op0=OP.mult)
                        nc.tensor.matmul(
                            pr[:], lhsT=selbig[:, t * 128:(t + 1) * 128],
                            rhs=xtc[:, t, :DC],
                            start=(t == 0), stop=(t == T - 1))
                    den = hot.tile([128, 1], F32, tag="den")
                    nc.vector.tensor_scalar(out=den[:], in0=pr[:, D:DC],
                                            scalar1=1e-30, scalar2=None,
                                            op0=OP.max)
                    rec = hot.tile([128, 1], F32, tag="rec")
                    nc.vector.reciprocal(rec[:], den[:])
                    rn = hot.tile([128, D], F32, tag="rn")
                    nc.vector.tensor_scalar(out=rn[:], in0=pr[:, :D],
                                            scalar1=rec[:, :1], scalar2=None,
                                            op0=OP.mult)
                    trans_to(r_hi, r_lo, rn, k)
                    if cond_pass:
                        icnt = hot.tile([128, 1], F32, tag="icnt")
                        nc.sync.dma_start(
                            out=icnt[:], in_=invcntg[k * 128:(k + 1) * 128, :])
                        cn = hot.tile([128, D], F32, tag="cn")
                        nc.vector.tensor_scalar(out=cn[:], in0=pc[:],
                                                scalar1=icnt[:, :1],
                                                scalar2=None, op0=OP.mult)
                        trans_to(condT_hi, condT_lo, cn, k)

            # ---------------- LSTM step 2 ----------------
            def lstm2(NSEG, r_hi, r_lo, pre, h_hi, h_lo, qn_d, with_aproj):
                for (s, w) in cfg.colchunks(NSEG):
                    rhi = sb1.tile([128, 512], F32, tag="l_rhi")
                    nc.sync.dma_start(out=rhi[:, :w], in_=r_hi[:, s:s + w])
                    rlo = sb1.tile([32, 512], F32, tag="l_rlo")
                    nc.sync.dma_start(out=rlo[:, :w], in_=r_lo[:, s:s + w])
                    gsb = {}
                    for gi_, (gn, fn) in enumerate(
                            [("i", AF.Sigmoid), ("f", AF.Sigmoid),
                             ("g", AF.Tanh), ("o", AF.Sigmoid)]):
                        WT_hi = prm[f"WreT{pre}{gi_}_hi"]
                        WT_lo = prm[f"WreT{pre}{gi_}_lo"]
                        for part, pn, p0, p1 in [("hi", 128, 0, 128),
                                                 ("lo", 32, 128, 160)]:
                            pg = psm.tile([pn, 512], F32, tag="m", space="PSUM")
                            nc.tensor.matmul(pg[:, :w], lhsT=WT_hi[:, p0:p1],
                                             rhs=rhi[:, :w], start=True,
                                             stop=False)
                            nc.tensor.matmul(pg[:, :w], lhsT=WT_lo[:, p0:p1],
                                             rhs=rlo[:, :w], start=False,
                                             stop=True)
                            g_ = sb1.tile([pn, 512], F32, tag=f"l_{gn}{part}")
                            nc.scalar.activation(
                                g_[:, :w], pg[:, :w], fn,
                                bias=prm[f"be2{pre}_{gn}_{part}"][:, :1])
                            gsb[gn + part] = g_
                    h2 = {}
                    for part, pn in [("hi", 128), ("lo", 32)]:
                        t1 = sb1.tile([pn, 512], F32, tag=f"l_t1{part}")
                        nc.vector.tensor_tensor(
                            out=t1[:, :w], in0=gsb["i" + part][:, :w],
                            in1=gsb["g" + part][:, :w], op=OP.mult)
                        c2 = sb1.tile([pn, 512], F32, tag=f"l_c2{part}")
                        nc.vector.scalar_tensor_tensor(
                            out=c2[:, :w], in0=gsb["f" + part][:, :w],
                            scalar=prm[f"c1{pre}_{part}"][:, :1],
                            in1=t1[:, :w], op0=OP.mult, op1=OP.add)
                        th = sb1.tile([pn, 512], F32, tag=f"l_th{part}")
                        nc.scalar.activation(th[:, :w], c2[:, :w], AF.Tanh)
                        hh = sb1.tile([pn, 512], F32, tag=f"l_hh{part}")
                        nc.vector.tensor_tensor(
                            out=hh[:, :w], in0=gsb["o" + part][:, :w],
                            in1=th[:, :w], op=OP.mult)
                        h2[part] = hh
                        nc.sync.dma_start(
                            out=(h_hi if part == "hi" else h_lo)[:, s:s + w],
                            in_=hh[:, :w])
                    if with_aproj:
                        qp = {}
                        for part, pn, p0, p1 in [("hi", 128, 0, 128),
                                                 ("lo", 32, 128, 160)]:
                            pq = psm.tile([pn, 512], F32, tag="m", space="PSUM")
                            nc.tensor.matmul(pq[:, :w],
                                             lhsT=prm["aw_hi"][:, p0:p1],
                                             rhs=h2["hi"][:, :w], start=True,
                                             stop=False)
                            nc.tensor.matmul(pq[:, :w],
                                             lhsT=prm["aw_lo"][:, p0:p1],
                                             rhs=h2["lo"][:, :w], start=False,
                                             stop=True)
                            qs = sb1.tile([pn, 512], F32, tag=f"l_qs{part}")
                            nc.vector.tensor_copy(out=qs[:, :w], in_=pq[:, :w])
                            qp[part] = qs
                        pe = psm.tile([1, 512], F32, tag="m", space="PSUM")
                        nc.tensor.matmul(pe[:, :w], lhsT=prm["ab_hi"][:, :1],
                                         rhs=h2["hi"][:, :w], start=True,
                                         stop=False)
                        nc.tensor.matmul(pe[:, :w], lhsT=prm["ab_lo"][:, :1],
                                         rhs=h2["lo"][:, :w], start=False,
                                         stop=True)
                        es = sb1.tile([1, 512], F32, tag="l_es")
                        nc.vector.tensor_copy(out=es[:, :w], in_=pe[:, :w])
                    else:
                        qp = {"hi": h2["hi"], "lo": h2["lo"]}
                        es = None
                    for j in range(w // 128):
                        k = (s + j * 128) // 128
                        qn_sb = hot.tile([128, DC], F32, tag="qnasm")
                        ta = psw.tile([128, 128], F32, tag="w", space="PSUM")
                        nc.tensor.transpose(
                            out=ta[:], in_=qp["hi"][:, j * 128:(j + 1) * 128],
                            identity=ident[:])
                        nc.vector.tensor_copy(out=qn_sb[:, :128], in_=ta[:])
                        tb = psw.tile([128, 32], F32, tag="w", space="PSUM")
                        nc.tensor.transpose(
                            out=tb[:], in_=qp["lo"][:, j * 128:(j + 1) * 128],
                            identity=ident[:32, :32])
                        nc.vector.tensor_copy(out=qn_sb[:, 128:160], in_=tb[:])
                        if es is not None:
                            tcq = psw.tile([128, 1], F32, tag="w", space="PSUM")
                            nc.tensor.transpose(
                                out=tcq[:], in_=es[:1, j * 128:(j + 1) * 128],
                                identity=ident[:1, :1])
                            nc.vector.tensor_copy(out=qn_sb[:, 160:161],
                                                  in_=tcq[:])
                        else:
                            nc.vector.memset(qn_sb[:, 160:161], 0.0)
                        nc.sync.dma_start(out=qn_d[k], in_=qn_sb[:])

            # ================= group side =================
            item_pass(KG, TG, a2g_idx, a2g_lc, xat, q1bA, xpackA, 1,
                      None, r1TA_hi, r1TA_lo, True, 2 * D + 1)
            lstm2(SGP, r1TA_hi, r1TA_lo, "A", h2TA_hi, h2TA_lo, qnA, True)
            item_pass(KG, TG, a2g_idx, a2g_lc, None, None, xpackA, 2,
                      qnA, r2TA_hi, r2TA_lo, False, DC)

            # ---- merge + film + gcn linears ----
            for (s, w) in cfg.colchunks(SGP):
                mrow = sb1.tile([1, 512], F32, tag="g_mrow")
                nc.sync.dma_start(out=mrow[:, :w], in_=maskg_row[:, s:s + w])
                mkb = sb1.tile([128, 512], F32, tag="g_mkb")
                nc.gpsimd.partition_broadcast(mkb[:, :w], mrow[:1, :w])
                h2s, r2s, conds = {}, {}, {}
                for part, pn in [("hi", 128), ("lo", 32)]:
                    h2s[part] = sb1.tile([pn, 512], F32, tag=f"g_h2{part}", name=f"g_h2{part}")
                    nc.sync.dma_start(
                        out=h2s[part][:, :w],
                        in_=(h2TA_hi if part == "hi" else h2TA_lo)[:, s:s + w])
                    r2s[part] = sb1.tile([pn, 512], F32, tag=f"g_r2{part}", name=f"g_r2{part}")
                    nc.sync.dma_start(
                        out=r2s[part][:, :w],
                        in_=(r2TA_hi if part == "hi" else r2TA_lo)[:, s:s + w])
                    conds[part] = sb1.tile([pn, 512], F32, tag=f"g_cs{part}", name=f"g_cs{part}")
                    nc.sync.dma_start(
                        out=conds[part][:, :w],
                        in_=(condT_hi if part == "hi" else condT_lo)[:, s:s + w])
                qm, rm = {}, {}
                for part, pn, p0, p1 in [("hi", 128, 0, 128),
                                         ("lo", 32, 128, 160)]:
                    pr2 = psm.tile([pn, 512], F32, tag="m", space="PSUM")
                    nc.tensor.matmul(pr2[:, :w], lhsT=prm["awT_hi"][:, p0:p1],
                                     rhs=r2s["hi"][:, :w], start=True,
                                     stop=False)
                    nc.tensor.matmul(pr2[:, :w], lhsT=prm["awT_lo"][:, p0:p1],
                                     rhs=r2s["lo"][:, :w], start=False,
                                     stop=True)
                    rt = sb1.tile([pn, 512], F32, tag=f"g_rt{part}")
                    nc.scalar.activation(rt[:, :w], pr2[:, :w], AF.Identity,
                                         bias=prm[f"ab_{part}"][:, :1])
                    rm[part] = sb1.tile([pn, 512], F32, tag=f"g_rm{part}", name=f"g_rm{part}")
                    nc.vector.tensor_tensor(out=rm[part][:, :w], in0=rt[:, :w],
                                            in1=mkb[:pn, :w], op=OP.mult)
                    qm[part] = sb1.tile([pn, 512], F32, tag=f"g_qm{part}", name=f"g_qm{part}")
                    nc.vector.tensor_tensor(out=qm[part][:, :w],
                                            in0=h2s[part][:, :w],
                                            in1=mkb[:pn, :w], op=OP.mult)
                # film hidden layers (shared across out slabs)
                t1s = {}
                for br in ("fg", "fb"):
                    for part, pn, p0, p1 in [("hi", 128, 0, 128),
                                             ("lo", 32, 128, 160)]:
                        p1t = psm.tile([pn, 512], F32, tag="m", space="PSUM")
                        nc.tensor.matmul(p1t[:, :w],
                                         lhsT=prm[f"{br}W1T_hi"][:, p0:p1],
                                         rhs=conds["hi"][:, :w], start=True,
                                         stop=False)
                        nc.tensor.matmul(p1t[:, :w],
                                         lhsT=prm[f"{br}W1T_lo"][:, p0:p1],
                                         rhs=conds["lo"][:, :w], start=False,
                                         stop=True)
                        t1sb = sb1.tile([pn, 512], F32, tag=f"g_t1{br}{part}")
                        nc.scalar.activation(
                            t1sb[:, :w], p1t[:, :w], AF.Relu,
                            bias=prm[f"{br}_b1_{part}"][:, :1])
                        t1s[br + part] = t1sb
                xgT = {}
                for part, pn, p0, p1 in [("hi", 128, 0, 128),
                                         ("lo", 32, 128, 160)]:
                    pf = psm.tile([pn, 512], F32, tag="m", space="PSUM")
                    nc.tensor.matmul(pf[:, :w], lhsT=prm["mwqT_hi"][:, p0:p1],
                                     rhs=qm["hi"][:, :w], start=True,
                                     stop=False)
                    nc.tensor.matmul(pf[:, :w], lhsT=prm["mwqT_lo"][:, p0:p1],
                                     rhs=qm["lo"][:, :w], start=False,
                                     stop=False)
                    nc.tensor.matmul(pf[:, :w], lhsT=prm["mwrT_hi"][:, p0:p1],
                                     rhs=rm["hi"][:, :w], start=False,
                                     stop=False)
                    nc.tensor.matmul(pf[:, :w], lhsT=prm["mwrT_lo"][:, p0:p1],
                                     rhs=rm["lo"][:, :w], start=False,
                                     stop=True)
                    xf = sb1.tile([pn, 512], F32, tag=f"g_xf{part}")
                    nc.scalar.activation(xf[:, :w], pf[:, :w], AF.Identity,
                                         bias=prm[f"mb_{part}"][:, :1])
                    gout = {}
                    for br in ("fg", "fb"):
                        p2t = psm.tile([pn, 512], F32, tag="m", space="PSUM")
                        nc.tensor.matmul(p2t[:, :w],
                                         lhsT=prm[f"{br}W2T_hi"][:, p0:p1],
                                         rhs=t1s[br + "hi"][:, :w],
                                         start=True, stop=False)
                        nc.tensor.matmul(p2t[:, :w],
                                         lhsT=prm[f"{br}W2T_lo"][:, p0:p1],
                                         rhs=t1s[br + "lo"][:, :w],
                                         start=False, stop=True)
                        gsb_ = sb1.tile([pn, 512], F32, tag=f"g_go{br}{part}")
                        nc.scalar.activation(
                            gsb_[:, :w], p2t[:, :w], AF.Identity,
                            bias=prm[f"{br}_b2_{part}"][:, :1])
                        gout[br] = gsb_
                    xgs = sb1.tile([pn, 512], F32, tag=f"g_xgs{part}")
                    nc.vector.tensor_tensor(out=xgs[:, :w],
                                            in0=gout["fg"][:, :w],
                                            in1=xf[:, :w], op=OP.mult)
                    nc.vector.tensor_tensor(out=xgs[:, :w], in0=xgs[:, :w],
                                            in1=gout["fb"][:, :w], op=OP.add)
                    xgT[part] = xgs
                pm = psm.tile([80, 512], F32, tag="m", space="PSUM")
                nc.tensor.matmul(pm[:, :w], lhsT=prm["msgT_hi"][:, :80],
                                 rhs=xgT["hi"][:, :w], start=True, stop=False)
                nc.tensor.matmul(pm[:, :w], lhsT=prm["msgT_lo"][:, :80],
                                 rhs=xgT["lo"][:, :w], start=False, stop=True)
                msb = sb1.tile([80, 512], F32, tag="g_msb")
                nc.scalar.activation(msb[:, :w], pm[:, :w], AF.Identity,
                                     bias=prm["msg_b"][:, :1])
                psf = psm.tile([80, 512], F32, tag="m", space="PSUM")
                nc.tensor.matmul(psf[:, :w], lhsT=prm["selfT_hi"][:, :80],
                                 rhs=xgT["hi"][:, :w], start=True, stop=False)
                nc.tensor.matmul(psf[:, :w], lhsT=prm["selfT_lo"][:, :80],
                                 rhs=xgT["lo"][:, :w], start=False, stop=True)
                ssb = sb1.tile([80, 512], F32, tag="g_ssb")
                nc.scalar.activation(ssb[:, :w], psf[:, :w], AF.Identity,
                                     bias=prm["self_b"][:, :1])
                nc.sync.dma_start(out=selfT_d[:, s:s + w], in_=ssb[:, :w])
                pu = psm.tile([1, 512], F32, tag="m", space="PSUM")
                nc.tensor.matmul(pu[:, :w], lhsT=prm["att"][:, :1],
                                 rhs=msb[:, :w], start=True, stop=True)
                usb = sb1.tile([1, 512], F32, tag="g_usb")
                nc.vector.tensor_copy(out=usb[:, :w], in_=pu[:, :w])
                xgr = sb1.tile([40, 512], F32, tag="g_xgr")
                nc.sync.dma_start(out=xgr[:, :w], in_=xgroupT[:, s:s + w])
                p0t = psm.tile([80, 512], F32, tag="m", space="PSUM")
                nc.tensor.matmul(p0t[:, :w], lhsT=prm["gwT"][:, :80],
                                 rhs=xgr[:, :w], start=True, stop=True)
                x0sb = sb1.tile([80, 512], F32, tag="g_x0sb")
                nc.scalar.activation(x0sb[:, :w], p0t[:, :w], AF.Identity,
                                     bias=prm["gb"][:, :1])
                nc.sync.dma_start(out=xg0T_d[:, s:s + w], in_=x0sb[:, :w])
                for j in range(w // 128):
                    k = (s + j * 128) // 128
                    man = hot.tile([128, 82], F32, tag="man")
                    tm = psw.tile([128, 80], F32, tag="w", space="PSUM")
                    nc.tensor.transpose(out=tm[:],
                                        in_=msb[:80, j * 128:(j + 1) * 128],
                                        identity=ident[:80, :80])
                    nc.vector.tensor_copy(out=man[:, :80], in_=tm[:])
                    nc.vector.memset(man[:, 80:81], 1.0)
                    tu = psw.tile([128, 1], F32, tag="w", space="PSUM")
                    nc.tensor.transpose(out=tu[:],
                                        in_=usb[:1, j * 128:(j + 1) * 128],
                                        identity=ident[:1, :1])
                    nc.vector.tensor_copy(out=man[:, 81:82], in_=tu[:])
                    nc.sync.dma_start(out=m_ag_in[k * 128:(k + 1) * 128, :],
                                      in_=man[:])

            nc.gpsimd.collective_compute(
                "AllGather", OP.bypass,
                replica_groups=[list(range(NCORES))],
                ins=[m_ag_in[:, :].opt()], outs=[m_full[:, :].opt()])

            # ---- edge pass ----
            for k in range(KE):
                elc = hot.tile([128, TE], F32, tag="elc")
                nc.sync.dma_start(out=elc[:], in_=edge_lc[k])
                eidx = hot.tile([128, TE], I32, tag="eidx")
                nc.sync.dma_start(out=eidx[:], in_=edge_idx[k])
                met = big.tile([128, TE, 82], F32, tag="met")
                for t in range(TE):
                    nc.gpsimd.indirect_dma_start(
                        out=met[:, t, :], out_offset=None, in_=m_full[:, :],
                        in_offset=bass.IndirectOffsetOnAxis(
                            ap=eidx[:, t:t + 1], axis=0))
                man_k = hot.tile([128, 82], F32, tag="mank")
                nc.sync.dma_start(out=man_k[:],
                                  in_=m_ag_in[k * 128:(k + 1) * 128, :])
                selbig = big.tile([128, TE * 128], F32, tag="selb")
                abuf = hot.tile([128, TE], F32, tag="abuf")
                for t in range(TE):
                    nc.vector.tensor_scalar(
                        out=selbig[:, t * 128:(t + 1) * 128], in0=iota[:],
                        scalar1=elc[:, t:t + 1], scalar2=None, op0=OP.is_equal)
                    stp = psw.tile([128, 128], F32, tag="w", space="PSUM")
                    nc.tensor.transpose(out=stp[:],
                                        in_=selbig[:, t * 128:(t + 1) * 128],
                                        identity=ident[:])
                    sts = hot.tile([128, 128], F32, tag="selTs")
                    nc.vector.tensor_copy(out=sts[:], in_=stp[:])
                    udp = psw.tile([128, 1], F32, tag="w", space="PSUM")
                    nc.tensor.matmul(udp[:], lhsT=sts[:], rhs=man_k[:, 81:82],
                                     start=True, stop=True)
                    nc.vector.tensor_tensor(out=abuf[:, t:t + 1],
                                            in0=met[:, t, 81:82], in1=udp[:],
                                            op=OP.add)
                lr = hot.tile([128, TE], F32, tag="lr")
                nc.scalar.activation(lr[:], abuf[:], AF.Lrelu, alpha=0.2)
                eea = hot.tile([128, TE], F32, tag="eea")
                nc.scalar.activation(eea[:], lr[:], AF.Exp)
                pE = psa.tile([128, 81], F32, tag="pr", space="PSUM")
                for t in range(TE):
                    nc.vector.tensor_scalar(
                        out=selbig[:, t * 128:(t + 1) * 128],
                        in0=selbig[:, t * 128:(t + 1) * 128],
                        scalar1=eea[:, t:t + 1], scalar2=None, op0=OP.mult)
                    nc.tensor.matmul(pE[:],
                                     lhsT=selbig[:, t * 128:(t + 1) * 128],
                                     rhs=met[:, t, :81],
                                     start=(t == 0), stop=(t == TE - 1))
                dE = hot.tile([128, 1], F32, tag="den")
                nc.vector.tensor_scalar(out=dE[:], in0=pE[:, 80:81],
                                        scalar1=1e-30, scalar2=None, op0=OP.max)
                rE = hot.tile([128, 1], F32, tag="rec")
                nc.vector.reciprocal(rE[:], dE[:])
                agg = hot.tile([128, 80], F32, tag="agg")
                nc.vector.tensor_scalar(out=agg[:], in0=pE[:, :80],
                                        scalar1=rE[:, :1], scalar2=None,
                                        op0=OP.mult)
                sfT = hot.tile([80, 128], F32, tag="sfT")
                nc.sync.dma_start(out=sfT[:],
                                  in_=selfT_d[:, k * 128:(k + 1) * 128])
                tsf = psw.tile([128, 80], F32, tag="w", space="PSUM")
                nc.tensor.transpose(out=tsf[:], in_=sfT[:], identity=ident[:80, :80])
                xgc = hot.tile([128, DC], F32, tag="xgc")
                nc.vector.tensor_tensor(out=xgc[:, 80:160], in0=agg[:],
                                        in1=tsf[:], op=OP.add)
                nc.scalar.activation(xgc[:, 80:160], xgc[:, 80:160], AF.Relu)
                x0T = hot.tile([80, 128], F32, tag="x0T")
                nc.sync.dma_start(out=x0T[:],
                                  in_=xg0T_d[:, k * 128:(k + 1) * 128])
                t0 = psw.tile([128, 80], F32, tag="w", space="PSUM")
                nc.tensor.transpose(out=t0[:], in_=x0T[:], identity=ident[:80, :80])
                nc.vector.tensor_copy(out=xgc[:, :80], in_=t0[:])
                nc.vector.memset(xgc[:, 160:161], 1.0)
                nc.sync.dma_start(out=xg_out[k * 128:(k + 1) * 128, :],
                                  in_=xgc[:, :160])
                nc.sync.dma_start(out=xg_ag_in[k * 128:(k + 1) * 128, :],
                                  in_=xgc[:])

            nc.gpsimd.collective_compute(
                "AllGather", OP.bypass,
                replica_groups=[list(range(NCORES))],
                ins=[xg_ag_in[:, :].opt()], outs=[xg_full[:, :].opt()])

            # ================= atom side =================
            item_pass(KA, TA, g2a_idx, g2a_lc, xg_full, q1bG, xpackG, 1,
                      None, r1TG_hi, r1TG_lo, False, DC)
            lstm2(SAP, r1TG_hi, r1TG_lo, "G", h2TG_hi, h2TG_lo, qnG, False)
            item_pass(KA, TA, g2a_idx, g2a_lc, None, None, xpackG, 2,
                      qnG, r2TG_hi, r2TG_lo, False, DC)

            # ---- final projection ----
            for (s, w) in cfg.colchunks(SAP):
                mrow = sb1.tile([1, 512], F32, tag="f_mrow")
                nc.sync.dma_start(out=mrow[:, :w], in_=maska_row[:, s:s + w])
                mkb = sb1.tile([128, 512], F32, tag="f_mkb")
                nc.gpsimd.partition_broadcast(mkb[:, :w], mrow[:1, :w])
                qm, rm = {}, {}
                for part, pn in [("hi", 128), ("lo", 32)]:
                    h2s = sb1.tile([pn, 512], F32, tag=f"f_h2{part}")
                    nc.sync.dma_start(
                        out=h2s[:, :w],
                        in_=(h2TG_hi if part == "hi" else h2TG_lo)[:, s:s + w])
                    r2s = sb1.tile([pn, 512], F32, tag=f"f_r2{part}")
                    nc.sync.dma_start(
                        out=r2s[:, :w],
                        in_=(r2TG_hi if part == "hi" else r2TG_lo)[:, s:s + w])
                    qm[part] = sb1.tile([pn, 512], F32, tag=f"f_qm{part}", name=f"f_qm{part}")
                    nc.vector.tensor_tensor(out=qm[part][:, :w],
                                            in0=h2s[:, :w],
                                            in1=mkb[:pn, :w], op=OP.mult)
                    rm[part] = sb1.tile([pn, 512], F32, tag=f"f_rm{part}", name=f"f_rm{part}")
                    nc.vector.tensor_tensor(out=rm[part][:, :w],
                                            in0=r2s[:, :w],
                                            in1=mkb[:pn, :w], op=OP.mult)
                for part, pn, p0, p1 in [("hi", 128, 0, 128),
                                         ("lo", 32, 128, 160)]:
                    pp_ = psm.tile([pn, 512], F32, tag="m", space="PSUM")
                    nc.tensor.matmul(pp_[:, :w], lhsT=prm["pwqT_hi"][:, p0:p1],
                                     rhs=qm["hi"][:, :w], start=True,
                                     stop=False)
                    nc.tensor.matmul(pp_[:, :w], lhsT=prm["pwqT_lo"][:, p0:p1],
                                     rhs=qm["lo"][:, :w], start=False,
                                     stop=False)
                    nc.tensor.matmul(pp_[:, :w], lhsT=prm["pwrT_hi"][:, p0:p1],
                                     rhs=rm["hi"][:, :w], start=False,
                                     stop=False)
                    nc.tensor.matmul(pp_[:, :w], lhsT=prm["pwrT_lo"][:, p0:p1],
                                     rhs=rm["lo"][:, :w], start=False,
                                     stop=True)
                    dsb = sb1.tile([pn, 512], F32, tag=f"f_d{part}")
                    nc.scalar.activation(dsb[:, :w], pp_[:, :w], AF.Identity,
                                         bias=prm[f"pb_{part}"][:, :1])
                    xas = sb1.tile([pn, 512], F32, tag=f"f_x{part}")
                    nc.sync.dma_start(
                        out=xas[:, :w],
                        in_=(xatomT_hi if part == "hi"
                             else xatomT_lo)[:, s:s + w])
                    nc.vector.tensor_tensor(out=dsb[:, :w], in0=dsb[:, :w],
                                            in1=xas[:, :w], op=OP.add)
                    nc.sync.dma_start(
                        out=(xaT_hi_o if part == "hi"
                             else xaT_lo_o)[:, s:s + w],
                        in_=dsb[:, :w])
    return nc


_CACHE = {}


def _get_compiled(cfg):
    key = (cfg.NA, cfg.G, cfg.NI, cfg.E, cfg.TG, cfg.TA, cfg.TE)
    if key in _CACHE:
        return _CACHE[key]
    nc = bacc.Bacc("TRN2", target_bir_lowering=False, debug=False,
                   enable_asserts=False, num_devices=NCORES)
    build(nc, cfg)
    nc.compile()
    _CACHE[key] = nc
    return nc


def kernel(**inputs):
    cfg = Cfg(NA=np.asarray(inputs["x_atom"]).shape[0],
              G=np.asarray(inputs["x_group"]).shape[0],
              NI=np.asarray(inputs["atom_idx"]).shape[0],
              E=np.asarray(inputs["edge_index_group"]).shape[1])
    in_maps = prepare(inputs, cfg)
    nc = _get_compiled(cfg)
    trace = os.environ.get("MESO_TRACE", "0") == "1"
    if trace:
        try:
            import hwprof
            hwprof.install()
        except Exception:
            trace = False
    res = bass_utils.run_bass_kernel_spmd(
        nc, in_maps, core_ids=list(range(NCORES)), trace=trace)
    LAST_EXEC_NS[0] = res.exec_time_ns
    xg_parts, xa_parts = [], []
    for c in range(NCORES):
        o = res.results[c]
        xg_parts.append(o["xg_out"][:cfg.SG])
        xaT = np.concatenate([o["xaT_hi"], o["xaT_lo"]], 0)
        xa_parts.append(np.ascontiguousarray(xaT.T[:cfg.SA]))
    return (np.concatenate(xa_parts, 0), np.concatenate(xg_parts, 0))
